# revision 2
# baseline (speedup 1.0000x reference)
"""v3: PE-offloaded ssq kernel. Per pair, the exp argument
-200*ssq = 400*sum_c win_c*ctr_c - 200*S2win - 200*S2ctr
is accumulated on the TensorEngine (5 matmuls per row j: 3x 400I@m_c,
-I@Gwin, -I@Gctr with G = 200*S2 precomputed), so the DVE only does the
3-channel product m = win*ctr, |dsal|, and P = wgt*|dsal|. The Act engine
reads the PSUM slot directly (exp), Pool does the sal window sub.
Mirror accumulation uses permutation-matrix matmuls (no psG/DMA shifts).

Layout: 120 partitions x 3 payload rows (global row 3p-6+j), per-channel
local window 13 rows x 372 cols fp16, all 4 channels in one tile.
PSUM: psA [120,3,512] (3 banks) + 5 rotating 1-bank slots for psX.
"""

import numpy as np

H = W = 352
RADIUS = 5
NP = 120                 # partitions; payload rows 3p-6 .. 3p-4
PADW2 = W + 20           # 372 : cols idx t <-> global col t-10
LROWS = 13               # local rows k <-> global row 3p-11+k
CH = LROWS * PADW2       # 4836 elements per channel
PW = W + 2 * RADIUS      # 362 : P/ssq domain, col q <-> global col q-5
N_CORES = 8

_CACHE = {}


def _build_kernel():
    from contextlib import ExitStack

    import concourse.bass as bass
    import concourse.tile as tile
    from concourse import bacc, mybir

    f16 = mybir.dt.float16
    f32 = mybir.dt.float32
    i16 = mybir.dt.int16
    Alu = mybir.AluOpType
    Act = mybir.ActivationFunctionType

    nc = bacc.Bacc(
        "TRN2",
        debug=False,
        enable_asserts=False,
        target_bir_lowering=False,
        num_devices=1,
        enable_partition_id=False,
    )
    # host-padded fp16 inputs: row r <-> global row r-11, col t <-> global t-10
    pred_d = nc.dram_tensor("pred16", [370, PADW2], f16, kind="ExternalInput")
    feat_d = nc.dram_tensor("feat16", [3, 370, PADW2], f16, kind="ExternalInput")
    out_d = nc.dram_tensor("partial", [NP, 4], f32, kind="ExternalOutput")

    with tile.TileContext(nc) as tc, ExitStack() as ctx:
        persist = ctx.enter_context(tc.tile_pool(name="persist", bufs=1))

        # all 4 channels in one tile
        ch4 = persist.tile([NP, 4, LROWS, PADW2], f16, tag="ch4")

        # rgb channels first; per-channel Square (Act) + odd-shift copy (DVE)
        # issued right after each channel's DMA so gsum/cho are ready early.
        sq = persist.tile([NP, 3, 8, PADW2], f16, tag="sq")
        gsum = persist.tile([NP, 8, PADW2], f16, tag="gsum")
        gtmp = persist.tile([NP, 8, PADW2], f16, tag="gtmp")
        # wave 1: local rows [a0, 10) per channel (enough for sy <= 2 pairs),
        # wave 2: rows [10, 13). Square/gsum follow per (channel, wave).
        def load_wave(c, a0, a1):
            src_ap = pred_d.ap() if c == 3 else feat_d.ap()[c]
            src = bass.AP(
                tensor=src_ap.tensor,
                offset=src_ap.offset + a0 * PADW2,
                ap=[[3 * PADW2, NP], [PADW2, a1 - a0], [1, PADW2]],
            )
            nc.sync.dma_start(out=ch4[:, c, a0:a1, :], in_=src)

        for c in (0, 1, 2, 3):
            load_wave(c, 5 if c < 3 else 3, 10)
            if c < 3:
                nc.scalar.activation(
                    out=sq[:, c, 0:5, :], in_=ch4[:, c, 5:10, :],
                    func=Act.Square, scale=14.142135623730951,
                )
            if c == 1:
                nc.vector.tensor_add(gtmp[:, 0:5], sq[:, 0, 0:5], sq[:, 1, 0:5])
        nc.vector.tensor_add(gsum[:, 0:5], gtmp[:, 0:5], sq[:, 2, 0:5])
        for c in (0, 1, 2, 3):
            load_wave(c, 10, LROWS)
            if c < 3:
                nc.scalar.activation(
                    out=sq[:, c, 5:8, :], in_=ch4[:, c, 10:13, :],
                    func=Act.Square, scale=14.142135623730951,
                )
            if c == 1:
                nc.vector.tensor_add(gtmp[:, 5:8], sq[:, 0, 5:8], sq[:, 1, 5:8])
        nc.vector.tensor_add(gsum[:, 5:8], gtmp[:, 5:8], sq[:, 2, 5:8])

        zeros = persist.tile([1, 5 * PADW2], f16, tag="zeros")
        nc.gpsimd.memset(zeros[:], 0.0)

        # ---- PE matrices: ident, 400*I, -I, perm1, perm2 ----
        ident = persist.tile([NP, NP], f16, tag="ident")
        i400 = persist.tile([NP, NP], f16, tag="i400")
        inegI = persist.tile([NP, NP], f16, tag="inegI")
        rowidx = persist.tile([NP, NP], i16, tag="rowidx")
        pidx = persist.tile([NP, 1], mybir.dt.int32, tag="pidx")
        pidxf = persist.tile([NP, 1], f32, tag="pidxf")
        nc.gpsimd.iota(rowidx[:], pattern=[[1, NP]], base=0, channel_multiplier=0)
        nc.gpsimd.iota(pidx[:], pattern=[[1, 1]], base=0, channel_multiplier=1)
        nc.vector.tensor_copy(out=pidxf[:], in_=pidx[:])
        nc.vector.tensor_scalar(
            out=ident[:], in0=rowidx[:], scalar1=pidxf[:], scalar2=None,
            op0=Alu.is_equal,
        )
        nc.vector.tensor_scalar(out=i400[:], in0=ident[:], scalar1=400.0,
                                scalar2=None, op0=Alu.mult)
        nc.vector.tensor_scalar(out=inegI[:], in0=ident[:], scalar1=-1.0,
                                scalar2=None, op0=Alu.mult)
        # perm_q[p, i] = 1 iff i == p + q  (matmul shifts source partition
        # p into target p+q)
        perms = {0: ident}
        for q in (1, 2):
            pq = persist.tile([NP, 1], f32, tag=f"pidxq{q}", name=f"pidxq{q}")
            nc.vector.tensor_scalar(out=pq[:], in0=pidxf[:], scalar1=float(q),
                                    scalar2=None, op0=Alu.add)
            pm = persist.tile([NP, NP], f16, tag=f"perm{q}", name=f"perm{q}")
            nc.vector.tensor_scalar(out=pm[:], in0=rowidx[:], scalar1=pq[:],
                                    scalar2=None, op0=Alu.is_equal)
            perms[q] = pm

        # ---- PSUM: psA accumulator (3 banks) + 5 rotating psX slots ----
        pp = ctx.enter_context(tc.tile_pool(name="ps", bufs=1, space="PSUM"))
        psA = pp.tile([NP, 3, 512], f32, tag="psA")
        psS = [pp.tile([NP, 512], f32, tag=f"psS{s}", name=f"psS{s}")
               for s in range(5)]

        tmp = ctx.enter_context(tc.tile_pool(name="tmp", bufs=3))

        # half set of shifts: sy in [1,5] all sx, plus sy=0 sx>0
        groups = [(sy, list(range(-5, 6))) for sy in (1, 2, 3, 4, 5)] + [
            (0, [sx for sx in range(1, 6)])
        ]
        pair_idx = 0
        unit = 0
        deferred = []

        def flush_deferred():
            # back-end of the previous pair: P-mul on DVE + 6 acc matmuls.
            # Emitted after the next pair's front-end so the DVE never
            # stalls waiting on the PE->Act chain of the current pair.
            if not deferred:
                return
            wgt_d, adsal_d, sy_d, sx_d, first = deferred.pop()
            P = tmp.tile([NP, 3, PW], f16, tag="P")
            nc.vector.tensor_mul(P[:], wgt_d[:], adsal_d[:])
            for j in range(3):
                nc.tensor.matmul(
                    out=psA[:, j, 0:PW], lhsT=ident[:], rhs=P[:, j, :],
                    start=first, stop=False,
                    skip_group_check=True,
                )
            for j in range(3):
                jp = (j - sy_d) % 3
                q = (jp + sy_d - j) // 3
                nc.tensor.matmul(
                    out=psA[:, j, RADIUS : RADIUS + W],
                    lhsT=perms[q][:],
                    rhs=P[:, jp, RADIUS - sx_d : RADIUS - sx_d + W],
                    start=False, stop=False,
                    skip_group_check=True,
                )

        for (sy, sxs) in groups:
            for sx in sxs:
                winr = ch4[:, 0:3, 5 + sy : 8 + sy, 5 + sx : 5 + sx + PW]
                ctr = ch4[:, 0:3, 5:8, 5 : 5 + PW]

                # F2-mix: some pairs compute q = 200*d^2 on Act (3 psX
                # matmuls) instead of m = win*ctr (5 psX matmuls), trading
                # idle Act time for PE time.
                use_f2 = pair_idx % 13 in (1, 3, 5, 8, 10)
                m = tmp.tile([NP, 3, 3, PW], f16, tag="m")
                if use_f2:
                    nc.vector.tensor_sub(m[:], winr, ctr)
                    qsq = tmp.tile([NP, 3, 3, PW], f16, tag="qsq")
                    nc.scalar.activation(out=qsq[:], in_=m[:],
                                         func=Act.Square,
                                         scale=14.142135623730951)
                else:
                    nc.vector.tensor_mul(m[:], winr, ctr)

                dsal = tmp.tile([NP, 3, PW], f16, tag="dsal")
                nc.gpsimd.tensor_sub(
                    dsal[:],
                    ch4[:, 3, 5 + sy : 8 + sy, 5 + sx : 5 + sx + PW],
                    ch4[:, 3, 5:8, 5 : 5 + PW],
                )
                adsal = tmp.tile([NP, 3, PW], f16, tag="adsal")
                nc.vector.tensor_scalar(
                    out=adsal[:].bitcast(mybir.dt.uint16),
                    in0=dsal[:].bitcast(mybir.dt.uint16),
                    scalar1=0x7FFF, scalar2=None, op0=Alu.bitwise_and,
                )

                # per-row ssq accumulation on PE + exp on Act
                wgt = tmp.tile([NP, 3, PW], f16, tag="wgt")
                s0 = unit % 5
                fused_exp = False  # fused exp needs one PSUM tile, which serializes deps
                for j in range(3):
                    slot = psS[unit % 5]
                    unit += 1
                    if use_f2:
                        # psX = -sum_c 200*d_c^2
                        for c in range(3):
                            nc.tensor.matmul(out=slot[:, 0:PW], lhsT=inegI[:],
                                             rhs=qsq[:, c, j, :],
                                             start=(c == 0), stop=(c == 2),
                                             skip_group_check=True)
                    else:
                        nc.tensor.matmul(out=slot[:, 0:PW], lhsT=i400[:],
                                         rhs=m[:, 0, j, :], start=True, stop=False,
                                         skip_group_check=True)
                        nc.tensor.matmul(out=slot[:, 0:PW], lhsT=i400[:],
                                         rhs=m[:, 1, j, :], start=False, stop=False,
                                         skip_group_check=True)
                        nc.tensor.matmul(out=slot[:, 0:PW], lhsT=i400[:],
                                         rhs=m[:, 2, j, :], start=False, stop=False,
                                         skip_group_check=True)
                        # -200*S2win : G row j+sy, cols shifted by sx
                        nc.tensor.matmul(out=slot[:, 0:PW], lhsT=inegI[:],
                                         rhs=gsum[:, j + sy, 5 + sx : 5 + sx + PW],
                                         start=False, stop=False,
                                         skip_group_check=True)
                        # -200*S2ctr : G row j, cols [5, 367)
                        nc.tensor.matmul(out=slot[:, 0:PW], lhsT=inegI[:],
                                         rhs=gsum[:, j, 5 : 5 + PW],
                                         start=False, stop=True,
                                         skip_group_check=True)
                    if not fused_exp:
                        nc.scalar.activation(out=wgt[:, j, :], in_=slot[:, 0:PW],
                                             func=Act.Exp, scale=1.0)


                cur = (wgt, adsal, sy, sx, pair_idx == 0)
                flush_deferred()
        mask = emit_mask()

        # ---- masked partial sums ----
        # mask-side reduce first (independent of psA, overlaps last pairs)
        sums = persist.tile([NP, 4], f32, tag="sums")
        mask2 = gtmp[:, 0:3, 0:PW]         # gtmp is dead after gsum
        nc.scalar.activation(
            out=mask2,
            in_=mask[:],
            func=Act.Identity, accum_out=sums[:, 1:2],
        )
        lm = persist.tile([NP, 3, PW], f16, tag="lm")
        scratch = persist.tile([NP, 3, PW], f16, tag="scratch")
        scratch2 = sq[:, 0, 0:3, 0:PW]     # sq is dead after gsum
        nc.vector.memset(sums[:, 2:4], 0.0)
        # per-j: evacuate psA bank, mask-multiply, accumulate on Act
        for j in range(3):
            nc.vector.tensor_copy(out=lm[:, j], in_=psA[:, j, 0:PW])
            nc.vector.tensor_mul(scratch[:, j], lm[:, j], mask[:, j])
            nc.scalar.activation(
                out=scratch2[:, j], in_=scratch[:, j],
                func=Act.Identity, accum_out=sums[:, 1 + j : 2 + j] if False else sums[:, 0:1] if j == 0 else (sums[:, 2:3] if j == 1 else sums[:, 3:4]),
            )
        # host sums partials; fold j-sums on host side via extra columns
        nc.sync.dma_start(out=out_d.ap(), in_=sums[:])

    nc.compile()
    return nc


def kernel(pred, feat):
    import os

    # A stale PJRT compilation-cache hit was observed to return a bad
    # executable (NaN result); force a fresh compile per process.
    os.environ.setdefault("JAX_ENABLE_COMPILATION_CACHE", "false")
    try:
        import jax

        jax.config.update("jax_enable_compilation_cache", False)
    except Exception:
        pass

    if "nc" not in _CACHE:
        _CACHE["nc"] = _build_kernel()
    nc = _CACHE["nc"]
    from concourse.bass_utils import run_bass_kernel_spmd

    pred = np.asarray(pred, dtype=np.float32).reshape(N_CORES, H, W)
    feat = np.asarray(feat, dtype=np.float32).reshape(N_CORES, 3, H, W)
    predp = np.zeros((N_CORES, 370, PADW2), np.float16)
    predp[:, 11:363, 10:362] = pred.astype(np.float16)
    featp = np.zeros((N_CORES, 3, 370, PADW2), np.float16)
    featp[:, :, 11:363, 10:362] = feat.astype(np.float16)
    in_maps = [
        {"pred16": np.ascontiguousarray(predp[i]),
         "feat16": np.ascontiguousarray(featp[i])}
        for i in range(N_CORES)
    ]
    res = run_bass_kernel_spmd(nc, in_maps, core_ids=list(range(N_CORES)))
    _CACHE["last_results"] = res
    tot = np.zeros(4, np.float64)
    for r in res.results:
        tot += r["partial"].astype(np.float64).sum(axis=0)
    loss = (tot[0] + tot[2] + tot[3]) / (tot[1] + 1e-6)
    return np.array(loss, dtype=np.float32)


# revision 3
# speedup vs baseline: 1.0018x; 1.0018x over previous
"""v3: PE-offloaded ssq kernel. Per pair, the exp argument
-200*ssq = 400*sum_c win_c*ctr_c - 200*S2win - 200*S2ctr
is accumulated on the TensorEngine (5 matmuls per row j: 3x 400I@m_c,
-I@Gwin, -I@Gctr with G = 200*S2 precomputed), so the DVE only does the
3-channel product m = win*ctr, |dsal|, and P = wgt*|dsal|. The Act engine
reads the PSUM slot directly (exp), Pool does the sal window sub.
Mirror accumulation uses permutation-matrix matmuls (no psG/DMA shifts).

Layout: 120 partitions x 3 payload rows (global row 3p-6+j), per-channel
local window 13 rows x 372 cols fp16, all 4 channels in one tile.
PSUM: psA [120,3,512] (3 banks) + 5 rotating 1-bank slots for psX.
"""

import numpy as np

H = W = 352
RADIUS = 5
NP = 120                 # partitions; payload rows 3p-6 .. 3p-4
PADW2 = W + 20           # 372 : cols idx t <-> global col t-10
LROWS = 13               # local rows k <-> global row 3p-11+k
CH = LROWS * PADW2       # 4836 elements per channel
PW = W + 2 * RADIUS      # 362 : P/ssq domain, col q <-> global col q-5
N_CORES = 8

_CACHE = {}


def _build_kernel():
    from contextlib import ExitStack

    import concourse.bass as bass
    import concourse.tile as tile
    from concourse import bacc, mybir

    f16 = mybir.dt.float16
    f32 = mybir.dt.float32
    i16 = mybir.dt.int16
    Alu = mybir.AluOpType
    Act = mybir.ActivationFunctionType

    nc = bacc.Bacc(
        "TRN2",
        debug=False,
        enable_asserts=False,
        target_bir_lowering=False,
        num_devices=1,
        enable_partition_id=False,
    )
    # host-padded fp16 inputs: row r <-> global row r-11, col t <-> global t-10
    pred_d = nc.dram_tensor("pred16", [370, PADW2], f16, kind="ExternalInput")
    feat_d = nc.dram_tensor("feat16", [3, 370, PADW2], f16, kind="ExternalInput")
    out_d = nc.dram_tensor("partial", [NP, 4], f32, kind="ExternalOutput")

    with tile.TileContext(nc) as tc, ExitStack() as ctx:
        persist = ctx.enter_context(tc.tile_pool(name="persist", bufs=1))

        # all 4 channels in one tile
        ch4 = persist.tile([NP, 4, LROWS, PADW2], f16, tag="ch4")

        # rgb channels first; per-channel Square (Act) + odd-shift copy (DVE)
        # issued right after each channel's DMA so gsum/cho are ready early.
        sq = persist.tile([NP, 3, 8, PADW2], f16, tag="sq")
        gsum = persist.tile([NP, 8, PADW2], f16, tag="gsum")
        gtmp = persist.tile([NP, 8, PADW2], f16, tag="gtmp")
        # wave 1: local rows [a0, 10) per channel (enough for sy <= 2 pairs),
        # wave 2: rows [10, 13). Square/gsum follow per (channel, wave).
        def load_wave(c, a0, a1):
            src_ap = pred_d.ap() if c == 3 else feat_d.ap()[c]
            src = bass.AP(
                tensor=src_ap.tensor,
                offset=src_ap.offset + a0 * PADW2,
                ap=[[3 * PADW2, NP], [PADW2, a1 - a0], [1, PADW2]],
            )
            nc.sync.dma_start(out=ch4[:, c, a0:a1, :], in_=src)

        for c in (0, 1, 2, 3):
            load_wave(c, 5 if c < 3 else 3, 9)
            if c < 3:
                nc.scalar.activation(
                    out=sq[:, c, 0:4, :], in_=ch4[:, c, 5:9, :],
                    func=Act.Square, scale=14.142135623730951,
                )
            if c == 1:
                nc.vector.tensor_add(gtmp[:, 0:4], sq[:, 0, 0:4], sq[:, 1, 0:4])
        nc.vector.tensor_add(gsum[:, 0:4], gtmp[:, 0:4], sq[:, 2, 0:4])
        for c in (0, 1, 2, 3):
            load_wave(c, 9, LROWS)
            if c < 3:
                nc.scalar.activation(
                    out=sq[:, c, 4:8, :], in_=ch4[:, c, 9:13, :],
                    func=Act.Square, scale=14.142135623730951,
                )
            if c == 1:
                nc.vector.tensor_add(gtmp[:, 4:8], sq[:, 0, 4:8], sq[:, 1, 4:8])
        nc.vector.tensor_add(gsum[:, 4:8], gtmp[:, 4:8], sq[:, 2, 4:8])

        zeros = persist.tile([1, 5 * PADW2], f16, tag="zeros")
        nc.gpsimd.memset(zeros[:], 0.0)

        # ---- PE matrices: ident, 400*I, -I, perm1, perm2 ----
        ident = persist.tile([NP, NP], f16, tag="ident")
        i400 = persist.tile([NP, NP], f16, tag="i400")
        inegI = persist.tile([NP, NP], f16, tag="inegI")
        rowidx = persist.tile([NP, NP], i16, tag="rowidx")
        pidx = persist.tile([NP, 1], mybir.dt.int32, tag="pidx")
        pidxf = persist.tile([NP, 1], f32, tag="pidxf")
        nc.gpsimd.iota(rowidx[:], pattern=[[1, NP]], base=0, channel_multiplier=0)
        nc.gpsimd.iota(pidx[:], pattern=[[1, 1]], base=0, channel_multiplier=1)
        nc.vector.tensor_copy(out=pidxf[:], in_=pidx[:])
        nc.vector.tensor_scalar(
            out=ident[:], in0=rowidx[:], scalar1=pidxf[:], scalar2=None,
            op0=Alu.is_equal,
        )
        nc.vector.tensor_scalar(out=i400[:], in0=ident[:], scalar1=400.0,
                                scalar2=None, op0=Alu.mult)
        nc.vector.tensor_scalar(out=inegI[:], in0=ident[:], scalar1=-1.0,
                                scalar2=None, op0=Alu.mult)
        # perm_q[p, i] = 1 iff i == p + q  (matmul shifts source partition
        # p into target p+q)
        perms = {0: ident}
        for q in (1, 2):
            pq = persist.tile([NP, 1], f32, tag=f"pidxq{q}", name=f"pidxq{q}")
            nc.vector.tensor_scalar(out=pq[:], in0=pidxf[:], scalar1=float(q),
                                    scalar2=None, op0=Alu.add)
            pm = persist.tile([NP, NP], f16, tag=f"perm{q}", name=f"perm{q}")
            nc.vector.tensor_scalar(out=pm[:], in0=rowidx[:], scalar1=pq[:],
                                    scalar2=None, op0=Alu.is_equal)
            perms[q] = pm

        # ---- PSUM: psA accumulator (3 banks) + 5 rotating psX slots ----
        pp = ctx.enter_context(tc.tile_pool(name="ps", bufs=1, space="PSUM"))
        psA = pp.tile([NP, 3, 512], f32, tag="psA")
        psS = [pp.tile([NP, 512], f32, tag=f"psS{s}", name=f"psS{s}")
               for s in range(5)]

        tmp = ctx.enter_context(tc.tile_pool(name="tmp", bufs=3))

        # half set of shifts: sy in [1,5] all sx, plus sy=0 sx>0
        groups = [(sy, list(range(-5, 6))) for sy in (1, 2, 3, 4, 5)] + [
            (0, [sx for sx in range(1, 6)])
        ]
        pair_idx = 0
        unit = 0
        deferred = []

        def flush_deferred():
            # back-end of the previous pair: P-mul on DVE + 6 acc matmuls.
            # Emitted after the next pair's front-end so the DVE never
            # stalls waiting on the PE->Act chain of the current pair.
            if not deferred:
                return
            wgt_d, adsal_d, sy_d, sx_d, first = deferred.pop()
            P = tmp.tile([NP, 3, PW], f16, tag="P")
            nc.vector.tensor_mul(P[:], wgt_d[:], adsal_d[:])
            for j in range(3):
                nc.tensor.matmul(
                    out=psA[:, j, 0:PW], lhsT=ident[:], rhs=P[:, j, :],
                    start=first, stop=False,
                    skip_group_check=True,
                )
            for j in range(3):
                jp = (j - sy_d) % 3
                q = (jp + sy_d - j) // 3
                nc.tensor.matmul(
                    out=psA[:, j, RADIUS : RADIUS + W],
                    lhsT=perms[q][:],
                    rhs=P[:, jp, RADIUS - sx_d : RADIUS - sx_d + W],
                    start=False, stop=False,
                    skip_group_check=True,
                )

        for (sy, sxs) in groups:
            for sx in sxs:
                winr = ch4[:, 0:3, 5 + sy : 8 + sy, 5 + sx : 5 + sx + PW]
                ctr = ch4[:, 0:3, 5:8, 5 : 5 + PW]

                # F2-mix: some pairs compute q = 200*d^2 on Act (3 psX
                # matmuls) instead of m = win*ctr (5 psX matmuls), trading
                # idle Act time for PE time.
                use_f2 = pair_idx % 13 in (1, 3, 5, 8, 10)
                m = tmp.tile([NP, 3, 3, PW], f16, tag="m")
                if use_f2:
                    nc.vector.tensor_sub(m[:], winr, ctr)
                    qsq = tmp.tile([NP, 3, 3, PW], f16, tag="qsq")
                    nc.scalar.activation(out=qsq[:], in_=m[:],
                                         func=Act.Square,
                                         scale=14.142135623730951)
                else:
                    nc.vector.tensor_mul(m[:], winr, ctr)

                dsal = tmp.tile([NP, 3, PW], f16, tag="dsal")
                nc.gpsimd.tensor_sub(
                    dsal[:],
                    ch4[:, 3, 5 + sy : 8 + sy, 5 + sx : 5 + sx + PW],
                    ch4[:, 3, 5:8, 5 : 5 + PW],
                )
                adsal = tmp.tile([NP, 3, PW], f16, tag="adsal")
                nc.vector.tensor_scalar(
                    out=adsal[:].bitcast(mybir.dt.uint16),
                    in0=dsal[:].bitcast(mybir.dt.uint16),
                    scalar1=0x7FFF, scalar2=None, op0=Alu.bitwise_and,
                )

                # per-row ssq accumulation on PE + exp on Act
                wgt = tmp.tile([NP, 3, PW], f16, tag="wgt")
                s0 = unit % 5
                fused_exp = False  # fused exp needs one PSUM tile, which serializes deps
                for j in range(3):
                    slot = psS[unit % 5]
                    unit += 1
                    if use_f2:
                        # psX = -sum_c 200*d_c^2
                        for c in range(3):
                            nc.tensor.matmul(out=slot[:, 0:PW], lhsT=inegI[:],
                                             rhs=qsq[:, c, j, :],
                                             start=(c == 0), stop=(c == 2),
                                             skip_group_check=True)
                    else:
                        nc.tensor.matmul(out=slot[:, 0:PW], lhsT=i400[:],
                                         rhs=m[:, 0, j, :], start=True, stop=False,
                                         skip_group_check=True)
                        nc.tensor.matmul(out=slot[:, 0:PW], lhsT=i400[:],
                                         rhs=m[:, 1, j, :], start=False, stop=False,
                                         skip_group_check=True)
                        nc.tensor.matmul(out=slot[:, 0:PW], lhsT=i400[:],
                                         rhs=m[:, 2, j, :], start=False, stop=False,
                                         skip_group_check=True)
                        # -200*S2win : G row j+sy, cols shifted by sx
                        nc.tensor.matmul(out=slot[:, 0:PW], lhsT=inegI[:],
                                         rhs=gsum[:, j + sy, 5 + sx : 5 + sx + PW],
                                         start=False, stop=False,
                                         skip_group_check=True)
                        # -200*S2ctr : G row j, cols [5, 367)
                        nc.tensor.matmul(out=slot[:, 0:PW], lhsT=inegI[:],
                                         rhs=gsum[:, j, 5 : 5 + PW],
                                         start=False, stop=True,
                                         skip_group_check=True)
                    if not fused_exp:
                        nc.scalar.activation(out=wgt[:, j, :], in_=slot[:, 0:PW],
                                             func=Act.Exp, scale=1.0)


                cur = (wgt, adsal, sy, sx, pair_idx == 0)
                flush_deferred()
        mask = emit_mask()

        # ---- masked partial sums ----
        # mask-side reduce first (independent of psA, overlaps last pairs)
        sums = persist.tile([NP, 4], f32, tag="sums")
        mask2 = gtmp[:, 0:3, 0:PW]         # gtmp is dead after gsum
        nc.scalar.activation(
            out=mask2,
            in_=mask[:],
            func=Act.Identity, accum_out=sums[:, 1:2],
        )
        lm = persist.tile([NP, 3, PW], f16, tag="lm")
        scratch = persist.tile([NP, 3, PW], f16, tag="scratch")
        scratch2 = sq[:, 0, 0:3, 0:PW]     # sq is dead after gsum
        nc.vector.memset(sums[:, 2:4], 0.0)
        # per-j: evacuate psA bank, mask-multiply, accumulate on Act
        for j in range(3):
            nc.vector.tensor_copy(out=lm[:, j], in_=psA[:, j, 0:PW])
            nc.vector.tensor_mul(scratch[:, j], lm[:, j], mask[:, j])
            nc.scalar.activation(
                out=scratch2[:, j], in_=scratch[:, j],
                func=Act.Identity, accum_out=sums[:, 1 + j : 2 + j] if False else sums[:, 0:1] if j == 0 else (sums[:, 2:3] if j == 1 else sums[:, 3:4]),
            )
        # host sums partials; fold j-sums on host side via extra columns
        nc.sync.dma_start(out=out_d.ap(), in_=sums[:])

    nc.compile()
    return nc


def kernel(pred, feat):
    import os

    # A stale PJRT compilation-cache hit was observed to return a bad
    # executable (NaN result); force a fresh compile per process.
    os.environ.setdefault("JAX_ENABLE_COMPILATION_CACHE", "false")
    try:
        import jax

        jax.config.update("jax_enable_compilation_cache", False)
    except Exception:
        pass

    if "nc" not in _CACHE:
        _CACHE["nc"] = _build_kernel()
    nc = _CACHE["nc"]
    from concourse.bass_utils import run_bass_kernel_spmd

    pred = np.asarray(pred, dtype=np.float32).reshape(N_CORES, H, W)
    feat = np.asarray(feat, dtype=np.float32).reshape(N_CORES, 3, H, W)
    predp = np.zeros((N_CORES, 370, PADW2), np.float16)
    predp[:, 11:363, 10:362] = pred.astype(np.float16)
    featp = np.zeros((N_CORES, 3, 370, PADW2), np.float16)
    featp[:, :, 11:363, 10:362] = feat.astype(np.float16)
    in_maps = [
        {"pred16": np.ascontiguousarray(predp[i]),
         "feat16": np.ascontiguousarray(featp[i])}
        for i in range(N_CORES)
    ]
    res = run_bass_kernel_spmd(nc, in_maps, core_ids=list(range(N_CORES)))
    _CACHE["last_results"] = res
    tot = np.zeros(4, np.float64)
    for r in res.results:
        tot += r["partial"].astype(np.float64).sum(axis=0)
    loss = (tot[0] + tot[2] + tot[3]) / (tot[1] + 1e-6)
    return np.array(loss, dtype=np.float32)


# revision 4
# speedup vs baseline: 1.0062x; 1.0045x over previous
"""v3: PE-offloaded ssq kernel. Per pair, the exp argument
-200*ssq = 400*sum_c win_c*ctr_c - 200*S2win - 200*S2ctr
is accumulated on the TensorEngine (5 matmuls per row j: 3x 400I@m_c,
-I@Gwin, -I@Gctr with G = 200*S2 precomputed), so the DVE only does the
3-channel product m = win*ctr, |dsal|, and P = wgt*|dsal|. The Act engine
reads the PSUM slot directly (exp), Pool does the sal window sub.
Mirror accumulation uses permutation-matrix matmuls (no psG/DMA shifts).

Layout: 120 partitions x 3 payload rows (global row 3p-6+j), per-channel
local window 13 rows x 372 cols fp16, all 4 channels in one tile.
PSUM: psA [120,3,512] (3 banks) + 5 rotating 1-bank slots for psX.
"""

import numpy as np

H = W = 352
RADIUS = 5
NP = 120                 # partitions; payload rows 3p-6 .. 3p-4
PADW2 = W + 20           # 372 : cols idx t <-> global col t-10
LROWS = 13               # local rows k <-> global row 3p-11+k
CH = LROWS * PADW2       # 4836 elements per channel
PW = W + 2 * RADIUS      # 362 : P/ssq domain, col q <-> global col q-5
N_CORES = 8

_CACHE = {}


def _build_kernel():
    from contextlib import ExitStack

    import concourse.bass as bass
    import concourse.tile as tile
    from concourse import bacc, mybir

    f16 = mybir.dt.float16
    f32 = mybir.dt.float32
    i16 = mybir.dt.int16
    Alu = mybir.AluOpType
    Act = mybir.ActivationFunctionType

    nc = bacc.Bacc(
        "TRN2",
        debug=False,
        enable_asserts=False,
        target_bir_lowering=False,
        num_devices=1,
        enable_partition_id=False,
    )
    # host-padded fp16 inputs: row r <-> global row r-11, col t <-> global t-10
    pred_d = nc.dram_tensor("pred16", [370, PADW2], f16, kind="ExternalInput")
    feat_d = nc.dram_tensor("feat16", [3, 370, PADW2], f16, kind="ExternalInput")
    out_d = nc.dram_tensor("partial", [NP, 4], f32, kind="ExternalOutput")

    with tile.TileContext(nc) as tc, ExitStack() as ctx:
        persist = ctx.enter_context(tc.tile_pool(name="persist", bufs=1))

        # all 4 channels in one tile
        ch4 = persist.tile([NP, 4, LROWS, PADW2], f16, tag="ch4")

        # rgb channels first; per-channel Square (Act) + odd-shift copy (DVE)
        # issued right after each channel's DMA so gsum/cho are ready early.
        sq = persist.tile([NP, 3, 8, PADW2], f16, tag="sq")
        gsum = persist.tile([NP, 8, PADW2], f16, tag="gsum")
        gtmp = persist.tile([NP, 8, PADW2], f16, tag="gtmp")
        # wave 1: local rows [a0, 10) per channel (enough for sy <= 2 pairs),
        # wave 2: rows [10, 13). Square/gsum follow per (channel, wave).
        def load_wave(c, a0, a1):
            src_ap = pred_d.ap() if c == 3 else feat_d.ap()[c]
            src = bass.AP(
                tensor=src_ap.tensor,
                offset=src_ap.offset + a0 * PADW2,
                ap=[[3 * PADW2, NP], [PADW2, a1 - a0], [1, PADW2]],
            )
            nc.sync.dma_start(out=ch4[:, c, a0:a1, :], in_=src)

        for c in (0, 1, 2, 3):
            load_wave(c, 5 if c < 3 else 3, 9)
            if c < 3:
                nc.scalar.activation(
                    out=sq[:, c, 0:4, :], in_=ch4[:, c, 5:9, :],
                    func=Act.Square, scale=14.142135623730951,
                )
            if c == 1:
                nc.vector.tensor_add(gtmp[:, 0:4], sq[:, 0, 0:4], sq[:, 1, 0:4])
        nc.vector.tensor_add(gsum[:, 0:4], gtmp[:, 0:4], sq[:, 2, 0:4])
        for c in (0, 1, 2, 3):
            load_wave(c, 9, LROWS)
            if c < 3:
                nc.scalar.activation(
                    out=sq[:, c, 4:8, :], in_=ch4[:, c, 9:13, :],
                    func=Act.Square, scale=14.142135623730951,
                )
            if c == 1:
                nc.vector.tensor_add(gtmp[:, 4:8], sq[:, 0, 4:8], sq[:, 1, 4:8])
        nc.vector.tensor_add(gsum[:, 4:8], gtmp[:, 4:8], sq[:, 2, 4:8])

        zeros = persist.tile([1, 5 * PADW2], f16, tag="zeros")
        nc.gpsimd.memset(zeros[:], 0.0)

        # ---- PE matrices: ident, 400*I, -I, perm1, perm2 ----
        ident = persist.tile([NP, NP], f16, tag="ident")
        i400 = persist.tile([NP, NP], f16, tag="i400")
        inegI = persist.tile([NP, NP], f16, tag="inegI")
        rowidx = persist.tile([NP, NP], i16, tag="rowidx")
        pidx = persist.tile([NP, 1], mybir.dt.int32, tag="pidx")
        pidxf = persist.tile([NP, 1], f32, tag="pidxf")
        nc.gpsimd.iota(rowidx[:], pattern=[[1, NP]], base=0, channel_multiplier=0)
        nc.gpsimd.iota(pidx[:], pattern=[[1, 1]], base=0, channel_multiplier=1)
        nc.vector.tensor_copy(out=pidxf[:], in_=pidx[:])
        nc.vector.tensor_scalar(
            out=ident[:], in0=rowidx[:], scalar1=pidxf[:], scalar2=None,
            op0=Alu.is_equal,
        )
        nc.vector.tensor_scalar(out=i400[:], in0=ident[:], scalar1=400.0,
                                scalar2=None, op0=Alu.mult)
        nc.vector.tensor_scalar(out=inegI[:], in0=ident[:], scalar1=-1.0,
                                scalar2=None, op0=Alu.mult)
        # perm_q[p, i] = 1 iff i == p + q  (matmul shifts source partition
        # p into target p+q)
        perms = {0: ident}
        for q in (1, 2):
            pq = persist.tile([NP, 1], f32, tag=f"pidxq{q}", name=f"pidxq{q}")
            nc.vector.tensor_scalar(out=pq[:], in0=pidxf[:], scalar1=float(q),
                                    scalar2=None, op0=Alu.add)
            pm = persist.tile([NP, NP], f16, tag=f"perm{q}", name=f"perm{q}")
            nc.vector.tensor_scalar(out=pm[:], in0=rowidx[:], scalar1=pq[:],
                                    scalar2=None, op0=Alu.is_equal)
            perms[q] = pm

        # ---- PSUM: psA accumulator (3 banks) + 5 rotating psX slots ----
        pp = ctx.enter_context(tc.tile_pool(name="ps", bufs=1, space="PSUM"))
        psA = pp.tile([NP, 3, 512], f32, tag="psA")
        psS = [pp.tile([NP, 512], f32, tag=f"psS{s}", name=f"psS{s}")
               for s in range(5)]

        tmp = ctx.enter_context(tc.tile_pool(name="tmp", bufs=4))

        # half set of shifts: sy in [1,5] all sx, plus sy=0 sx>0
        groups = [(sy, list(range(-5, 6))) for sy in (1, 2, 3, 4, 5)] + [
            (0, [sx for sx in range(1, 6)])
        ]
        pair_idx = 0
        unit = 0
        deferred = []

        def flush_deferred():
            # back-end of the previous pair: P-mul on DVE + 6 acc matmuls.
            # Emitted after the next pair's front-end so the DVE never
            # stalls waiting on the PE->Act chain of the current pair.
            if not deferred:
                return
            wgt_d, adsal_d, sy_d, sx_d, first = deferred.pop()
            P = tmp.tile([NP, 3, PW], f16, tag="P")
            nc.vector.tensor_mul(P[:], wgt_d[:], adsal_d[:])
            for j in range(3):
                nc.tensor.matmul(
                    out=psA[:, j, 0:PW], lhsT=ident[:], rhs=P[:, j, :],
                    start=first, stop=False,
                    skip_group_check=True,
                )
            for j in range(3):
                jp = (j - sy_d) % 3
                q = (jp + sy_d - j) // 3
                nc.tensor.matmul(
                    out=psA[:, j, RADIUS : RADIUS + W],
                    lhsT=perms[q][:],
                    rhs=P[:, jp, RADIUS - sx_d : RADIUS - sx_d + W],
                    start=False, stop=False,
                    skip_group_check=True,
                )

        for (sy, sxs) in groups:
            for sx in sxs:
                winr = ch4[:, 0:3, 5 + sy : 8 + sy, 5 + sx : 5 + sx + PW]
                ctr = ch4[:, 0:3, 5:8, 5 : 5 + PW]

                # F2-mix: some pairs compute q = 200*d^2 on Act (3 psX
                # matmuls) instead of m = win*ctr (5 psX matmuls), trading
                # idle Act time for PE time.
                use_f2 = pair_idx % 13 in (1, 3, 5, 8, 10)
                m = tmp.tile([NP, 3, 3, PW], f16, tag="m")
                if use_f2:
                    nc.vector.tensor_sub(m[:], winr, ctr)
                    qsq = tmp.tile([NP, 3, 3, PW], f16, tag="qsq")
                    nc.scalar.activation(out=qsq[:], in_=m[:],
                                         func=Act.Square,
                                         scale=14.142135623730951)
                else:
                    nc.vector.tensor_mul(m[:], winr, ctr)

                dsal = tmp.tile([NP, 3, PW], f16, tag="dsal")
                nc.gpsimd.tensor_sub(
                    dsal[:],
                    ch4[:, 3, 5 + sy : 8 + sy, 5 + sx : 5 + sx + PW],
                    ch4[:, 3, 5:8, 5 : 5 + PW],
                )
                adsal = tmp.tile([NP, 3, PW], f16, tag="adsal")
                nc.vector.tensor_scalar(
                    out=adsal[:].bitcast(mybir.dt.uint16),
                    in0=dsal[:].bitcast(mybir.dt.uint16),
                    scalar1=0x7FFF, scalar2=None, op0=Alu.bitwise_and,
                )

                # per-row ssq accumulation on PE + exp on Act
                wgt = tmp.tile([NP, 3, PW], f16, tag="wgt")
                s0 = unit % 5
                fused_exp = False  # fused exp needs one PSUM tile, which serializes deps
                for j in range(3):
                    slot = psS[unit % 5]
                    unit += 1
                    if use_f2:
                        # psX = -sum_c 200*d_c^2
                        for c in range(3):
                            nc.tensor.matmul(out=slot[:, 0:PW], lhsT=inegI[:],
                                             rhs=qsq[:, c, j, :],
                                             start=(c == 0), stop=(c == 2),
                                             skip_group_check=True)
                    else:
                        nc.tensor.matmul(out=slot[:, 0:PW], lhsT=i400[:],
                                         rhs=m[:, 0, j, :], start=True, stop=False,
                                         skip_group_check=True)
                        nc.tensor.matmul(out=slot[:, 0:PW], lhsT=i400[:],
                                         rhs=m[:, 1, j, :], start=False, stop=False,
                                         skip_group_check=True)
                        nc.tensor.matmul(out=slot[:, 0:PW], lhsT=i400[:],
                                         rhs=m[:, 2, j, :], start=False, stop=False,
                                         skip_group_check=True)
                        # -200*S2win : G row j+sy, cols shifted by sx
                        nc.tensor.matmul(out=slot[:, 0:PW], lhsT=inegI[:],
                                         rhs=gsum[:, j + sy, 5 + sx : 5 + sx + PW],
                                         start=False, stop=False,
                                         skip_group_check=True)
                        # -200*S2ctr : G row j, cols [5, 367)
                        nc.tensor.matmul(out=slot[:, 0:PW], lhsT=inegI[:],
                                         rhs=gsum[:, j, 5 : 5 + PW],
                                         start=False, stop=True,
                                         skip_group_check=True)
                    if not fused_exp:
                        nc.scalar.activation(out=wgt[:, j, :], in_=slot[:, 0:PW],
                                             func=Act.Exp, scale=1.0)


                cur = (wgt, adsal, sy, sx, pair_idx == 0)
                flush_deferred()
        mask = emit_mask()

        # ---- masked partial sums ----
        # mask-side reduce first (independent of psA, overlaps last pairs)
        sums = persist.tile([NP, 4], f32, tag="sums")
        mask2 = gtmp[:, 0:3, 0:PW]         # gtmp is dead after gsum
        nc.scalar.activation(
            out=mask2,
            in_=mask[:],
            func=Act.Identity, accum_out=sums[:, 1:2],
        )
        lm = persist.tile([NP, 3, PW], f16, tag="lm")
        scratch = persist.tile([NP, 3, PW], f16, tag="scratch")
        scratch2 = sq[:, 0, 0:3, 0:PW]     # sq is dead after gsum
        nc.vector.memset(sums[:, 2:4], 0.0)
        # per-j: evacuate psA bank, mask-multiply, accumulate on Act
        for j in range(3):
            nc.vector.tensor_copy(out=lm[:, j], in_=psA[:, j, 0:PW])
            nc.vector.tensor_mul(scratch[:, j], lm[:, j], mask[:, j])
            nc.scalar.activation(
                out=scratch2[:, j], in_=scratch[:, j],
                func=Act.Identity, accum_out=sums[:, 1 + j : 2 + j] if False else sums[:, 0:1] if j == 0 else (sums[:, 2:3] if j == 1 else sums[:, 3:4]),
            )
        # host sums partials; fold j-sums on host side via extra columns
        nc.sync.dma_start(out=out_d.ap(), in_=sums[:])

    nc.compile()
    return nc


def kernel(pred, feat):
    import os

    # A stale PJRT compilation-cache hit was observed to return a bad
    # executable (NaN result); force a fresh compile per process.
    os.environ.setdefault("JAX_ENABLE_COMPILATION_CACHE", "false")
    try:
        import jax

        jax.config.update("jax_enable_compilation_cache", False)
    except Exception:
        pass

    if "nc" not in _CACHE:
        _CACHE["nc"] = _build_kernel()
    nc = _CACHE["nc"]
    from concourse.bass_utils import run_bass_kernel_spmd

    pred = np.asarray(pred, dtype=np.float32).reshape(N_CORES, H, W)
    feat = np.asarray(feat, dtype=np.float32).reshape(N_CORES, 3, H, W)
    predp = np.zeros((N_CORES, 370, PADW2), np.float16)
    predp[:, 11:363, 10:362] = pred.astype(np.float16)
    featp = np.zeros((N_CORES, 3, 370, PADW2), np.float16)
    featp[:, :, 11:363, 10:362] = feat.astype(np.float16)
    in_maps = [
        {"pred16": np.ascontiguousarray(predp[i]),
         "feat16": np.ascontiguousarray(featp[i])}
        for i in range(N_CORES)
    ]
    res = run_bass_kernel_spmd(nc, in_maps, core_ids=list(range(N_CORES)))
    _CACHE["last_results"] = res
    tot = np.zeros(4, np.float64)
    for r in res.results:
        tot += r["partial"].astype(np.float64).sum(axis=0)
    loss = (tot[0] + tot[2] + tot[3]) / (tot[1] + 1e-6)
    return np.array(loss, dtype=np.float32)


# revision 5
# speedup vs baseline: 1.0078x; 1.0016x over previous
"""v3: PE-offloaded ssq kernel. Per pair, the exp argument
-200*ssq = 400*sum_c win_c*ctr_c - 200*S2win - 200*S2ctr
is accumulated on the TensorEngine (5 matmuls per row j: 3x 400I@m_c,
-I@Gwin, -I@Gctr with G = 200*S2 precomputed), so the DVE only does the
3-channel product m = win*ctr, |dsal|, and P = wgt*|dsal|. The Act engine
reads the PSUM slot directly (exp), Pool does the sal window sub.
Mirror accumulation uses permutation-matrix matmuls (no psG/DMA shifts).

Layout: 120 partitions x 3 payload rows (global row 3p-6+j), per-channel
local window 13 rows x 372 cols fp16, all 4 channels in one tile.
PSUM: psA [120,3,512] (3 banks) + 5 rotating 1-bank slots for psX.
"""

import numpy as np

H = W = 352
RADIUS = 5
NP = 120                 # partitions; payload rows 3p-6 .. 3p-4
PADW2 = W + 20           # 372 : cols idx t <-> global col t-10
LROWS = 13               # local rows k <-> global row 3p-11+k
CH = LROWS * PADW2       # 4836 elements per channel
PW = W + 2 * RADIUS      # 362 : P/ssq domain, col q <-> global col q-5
N_CORES = 8

_CACHE = {}


def _build_kernel():
    from contextlib import ExitStack

    import concourse.bass as bass
    import concourse.tile as tile
    from concourse import bacc, mybir

    f16 = mybir.dt.float16
    f32 = mybir.dt.float32
    i16 = mybir.dt.int16
    Alu = mybir.AluOpType
    Act = mybir.ActivationFunctionType

    nc = bacc.Bacc(
        "TRN2",
        debug=False,
        enable_asserts=False,
        target_bir_lowering=False,
        num_devices=1,
        enable_partition_id=False,
    )
    # host-padded fp16 inputs: row r <-> global row r-11, col t <-> global t-10
    pred_d = nc.dram_tensor("pred16", [370, PADW2], f16, kind="ExternalInput")
    feat_d = nc.dram_tensor("feat16", [3, 370, PADW2], f16, kind="ExternalInput")
    out_d = nc.dram_tensor("partial", [NP, 4], f32, kind="ExternalOutput")

    with tile.TileContext(nc) as tc, ExitStack() as ctx:
        persist = ctx.enter_context(tc.tile_pool(name="persist", bufs=1))

        # all 4 channels in one tile
        ch4 = persist.tile([NP, 4, LROWS, PADW2], f16, tag="ch4")

        # rgb channels first; per-channel Square (Act) + odd-shift copy (DVE)
        # issued right after each channel's DMA so gsum/cho are ready early.
        sq = persist.tile([NP, 3, 8, PADW2], f16, tag="sq")
        gsum = persist.tile([NP, 8, PADW2], f16, tag="gsum")
        gtmp = persist.tile([NP, 8, PADW2], f16, tag="gtmp")
        # wave 1: local rows [a0, 10) per channel (enough for sy <= 2 pairs),
        # wave 2: rows [10, 13). Square/gsum follow per (channel, wave).
        def load_wave(c, a0, a1):
            src_ap = pred_d.ap() if c == 3 else feat_d.ap()[c]
            src = bass.AP(
                tensor=src_ap.tensor,
                offset=src_ap.offset + a0 * PADW2,
                ap=[[3 * PADW2, NP], [PADW2, a1 - a0], [1, PADW2]],
            )
            nc.sync.dma_start(out=ch4[:, c, a0:a1, :], in_=src)

        # wave 0: rows [5,8) (all sy=0 pairs); wave 1: [8,10) (sy<=2);
        # wave 2: [10,13). sal rows [5,8) early for sy=0 dsal, rest later.
        waves = [(5, 8, 0, 3), (8, 10, 3, 5), (10, 13, 5, 8)]
        for wi, (a0, a1, k0, k1) in enumerate(waves):
            for c in (0, 1, 2):
                load_wave(c, a0, a1)
                nc.scalar.activation(
                    out=sq[:, c, k0:k1, :], in_=ch4[:, c, a0:a1, :],
                    func=Act.Square, scale=14.142135623730951,
                )
            if wi == 0:
                load_wave(3, 5, 8)
            elif wi == 1:
                load_wave(3, 8, 10)
            nc.vector.tensor_add(gtmp[:, k0:k1], sq[:, 0, k0:k1], sq[:, 1, k0:k1])
            nc.vector.tensor_add(gsum[:, k0:k1], gtmp[:, k0:k1], sq[:, 2, k0:k1])
        load_wave(3, 3, 5)
        load_wave(3, 10, LROWS)

        zeros = persist.tile([1, 5 * PADW2], f16, tag="zeros")
        nc.gpsimd.memset(zeros[:], 0.0)

        # ---- PE matrices: ident, 400*I, -I, perm1, perm2 ----
        ident = persist.tile([NP, NP], f16, tag="ident")
        i400 = persist.tile([NP, NP], f16, tag="i400")
        inegI = persist.tile([NP, NP], f16, tag="inegI")
        rowidx = persist.tile([NP, NP], i16, tag="rowidx")
        pidx = persist.tile([NP, 1], mybir.dt.int32, tag="pidx")
        pidxf = persist.tile([NP, 1], f32, tag="pidxf")
        nc.gpsimd.iota(rowidx[:], pattern=[[1, NP]], base=0, channel_multiplier=0)
        nc.gpsimd.iota(pidx[:], pattern=[[1, 1]], base=0, channel_multiplier=1)
        nc.vector.tensor_copy(out=pidxf[:], in_=pidx[:])
        nc.vector.tensor_scalar(
            out=ident[:], in0=rowidx[:], scalar1=pidxf[:], scalar2=None,
            op0=Alu.is_equal,
        )
        nc.vector.tensor_scalar(out=i400[:], in0=ident[:], scalar1=400.0,
                                scalar2=None, op0=Alu.mult)
        nc.vector.tensor_scalar(out=inegI[:], in0=ident[:], scalar1=-1.0,
                                scalar2=None, op0=Alu.mult)
        # perm_q[p, i] = 1 iff i == p + q  (matmul shifts source partition
        # p into target p+q)
        perms = {0: ident}
        for q in (1, 2):
            pq = persist.tile([NP, 1], f32, tag=f"pidxq{q}", name=f"pidxq{q}")
            nc.vector.tensor_scalar(out=pq[:], in0=pidxf[:], scalar1=float(q),
                                    scalar2=None, op0=Alu.add)
            pm = persist.tile([NP, NP], f16, tag=f"perm{q}", name=f"perm{q}")
            nc.vector.tensor_scalar(out=pm[:], in0=rowidx[:], scalar1=pq[:],
                                    scalar2=None, op0=Alu.is_equal)
            perms[q] = pm

        # ---- PSUM: psA accumulator (3 banks) + 5 rotating psX slots ----
        pp = ctx.enter_context(tc.tile_pool(name="ps", bufs=1, space="PSUM"))
        psA = pp.tile([NP, 3, 512], f32, tag="psA")
        psS = [pp.tile([NP, 512], f32, tag=f"psS{s}", name=f"psS{s}")
               for s in range(5)]

        tmp = ctx.enter_context(tc.tile_pool(name="tmp", bufs=4))

        # half set of shifts: sy in [1,5] all sx, plus sy=0 sx>0
        groups = [(0, [sx for sx in range(1, 6)])] + [
            (sy, list(range(-5, 6))) for sy in (1, 2, 3, 4, 5)
        ]
        pair_idx = 0
        unit = 0
        deferred = []

        def flush_deferred():
            # back-end of the previous pair: P-mul on DVE + 6 acc matmuls.
            # Emitted after the next pair's front-end so the DVE never
            # stalls waiting on the PE->Act chain of the current pair.
            if not deferred:
                return
            wgt_d, adsal_d, sy_d, sx_d, first = deferred.pop()
            P = tmp.tile([NP, 3, PW], f16, tag="P")
            nc.vector.tensor_mul(P[:], wgt_d[:], adsal_d[:])
            for j in range(3):
                nc.tensor.matmul(
                    out=psA[:, j, 0:PW], lhsT=ident[:], rhs=P[:, j, :],
                    start=first, stop=False,
                    skip_group_check=True,
                )
            for j in range(3):
                jp = (j - sy_d) % 3
                q = (jp + sy_d - j) // 3
                nc.tensor.matmul(
                    out=psA[:, j, RADIUS : RADIUS + W],
                    lhsT=perms[q][:],
                    rhs=P[:, jp, RADIUS - sx_d : RADIUS - sx_d + W],
                    start=False, stop=False,
                    skip_group_check=True,
                )

        for (sy, sxs) in groups:
            for sx in sxs:
                winr = ch4[:, 0:3, 5 + sy : 8 + sy, 5 + sx : 5 + sx + PW]
                ctr = ch4[:, 0:3, 5:8, 5 : 5 + PW]

                # F2-mix: some pairs compute q = 200*d^2 on Act (3 psX
                # matmuls) instead of m = win*ctr (5 psX matmuls), trading
                # idle Act time for PE time.
                use_f2 = pair_idx % 13 in (1, 3, 5, 8, 10)
                m = tmp.tile([NP, 3, 3, PW], f16, tag="m")
                if use_f2:
                    nc.vector.tensor_sub(m[:], winr, ctr)
                    qsq = tmp.tile([NP, 3, 3, PW], f16, tag="qsq")
                    nc.scalar.activation(out=qsq[:], in_=m[:],
                                         func=Act.Square,
                                         scale=14.142135623730951)
                else:
                    nc.vector.tensor_mul(m[:], winr, ctr)

                dsal = tmp.tile([NP, 3, PW], f16, tag="dsal")
                nc.gpsimd.tensor_sub(
                    dsal[:],
                    ch4[:, 3, 5 + sy : 8 + sy, 5 + sx : 5 + sx + PW],
                    ch4[:, 3, 5:8, 5 : 5 + PW],
                )
                adsal = tmp.tile([NP, 3, PW], f16, tag="adsal")
                nc.vector.tensor_scalar(
                    out=adsal[:].bitcast(mybir.dt.uint16),
                    in0=dsal[:].bitcast(mybir.dt.uint16),
                    scalar1=0x7FFF, scalar2=None, op0=Alu.bitwise_and,
                )

                # per-row ssq accumulation on PE + exp on Act
                wgt = tmp.tile([NP, 3, PW], f16, tag="wgt")
                for j in range(3):
                    slot = psS[unit % 5]
                    unit += 1
                    if use_f2:
                        # psX = -sum_c 200*d_c^2
                        for c in range(3):
                            nc.tensor.matmul(out=slot[:, 0:PW], lhsT=inegI[:],
                                             rhs=qsq[:, c, j, :],
                                             start=(c == 0), stop=(c == 2),
                                             skip_group_check=True)
                    else:
                        nc.tensor.matmul(out=slot[:, 0:PW], lhsT=i400[:],
                                         rhs=m[:, 0, j, :], start=True, stop=False,
                                         skip_group_check=True)
                        nc.tensor.matmul(out=slot[:, 0:PW], lhsT=i400[:],
                                         rhs=m[:, 1, j, :], start=False, stop=False,
                                         skip_group_check=True)
                        nc.tensor.matmul(out=slot[:, 0:PW], lhsT=i400[:],
                                         rhs=m[:, 2, j, :], start=False, stop=False,
                                         skip_group_check=True)
                        # -200*S2win : G row j+sy, cols shifted by sx
                        nc.tensor.matmul(out=slot[:, 0:PW], lhsT=inegI[:],
                                         rhs=gsum[:, j + sy, 5 + sx : 5 + sx + PW],
                                         start=False, stop=False,
                                         skip_group_check=True)
                        # -200*S2ctr : G row j, cols [5, 367)
                        nc.tensor.matmul(out=slot[:, 0:PW], lhsT=inegI[:],
                                         rhs=gsum[:, j, 5 : 5 + PW],
                                         start=False, stop=True,
                                         skip_group_check=True)
                    nc.scalar.activation(out=wgt[:, j, :], in_=slot[:, 0:PW],
                                         func=Act.Exp, scale=1.0)


                cur = (wgt, adsal, sy, sx, pair_idx == 0)
                flush_deferred()
        mask = emit_mask()

        # ---- masked partial sums ----
        # mask-side reduce first (independent of psA, overlaps last pairs)
        sums = persist.tile([NP, 4], f32, tag="sums")
        mask2 = gtmp[:, 0:3, 0:PW]         # gtmp is dead after gsum
        nc.scalar.activation(
            out=mask2,
            in_=mask[:],
            func=Act.Identity, accum_out=sums[:, 1:2],
        )
        lm = persist.tile([NP, 3, PW], f16, tag="lm")
        scratch = persist.tile([NP, 3, PW], f16, tag="scratch")
        scratch2 = sq[:, 0, 0:3, 0:PW]     # sq is dead after gsum
        nc.vector.memset(sums[:, 2:4], 0.0)
        # per-j: evacuate psA bank, mask-multiply, accumulate on Act
        for j in range(3):
            nc.vector.tensor_copy(out=lm[:, j], in_=psA[:, j, 0:PW])
            nc.vector.tensor_mul(scratch[:, j], lm[:, j], mask[:, j])
            nc.scalar.activation(
                out=scratch2[:, j], in_=scratch[:, j],
                func=Act.Identity, accum_out=sums[:, 1 + j : 2 + j] if False else sums[:, 0:1] if j == 0 else (sums[:, 2:3] if j == 1 else sums[:, 3:4]),
            )
        # host sums partials; fold j-sums on host side via extra columns
        nc.sync.dma_start(out=out_d.ap(), in_=sums[:])

    nc.compile()
    return nc


def kernel(pred, feat):
    import os

    # A stale PJRT compilation-cache hit was observed to return a bad
    # executable (NaN result); force a fresh compile per process.
    os.environ.setdefault("JAX_ENABLE_COMPILATION_CACHE", "false")
    try:
        import jax

        jax.config.update("jax_enable_compilation_cache", False)
    except Exception:
        pass

    if "nc" not in _CACHE:
        _CACHE["nc"] = _build_kernel()
    nc = _CACHE["nc"]
    from concourse.bass_utils import run_bass_kernel_spmd

    pred = np.asarray(pred, dtype=np.float32).reshape(N_CORES, H, W)
    feat = np.asarray(feat, dtype=np.float32).reshape(N_CORES, 3, H, W)
    predp = np.zeros((N_CORES, 370, PADW2), np.float16)
    predp[:, 11:363, 10:362] = pred.astype(np.float16)
    featp = np.zeros((N_CORES, 3, 370, PADW2), np.float16)
    featp[:, :, 11:363, 10:362] = feat.astype(np.float16)
    in_maps = [
        {"pred16": np.ascontiguousarray(predp[i]),
         "feat16": np.ascontiguousarray(featp[i])}
        for i in range(N_CORES)
    ]
    res = run_bass_kernel_spmd(nc, in_maps, core_ids=list(range(N_CORES)))
    _CACHE["last_results"] = res
    tot = np.zeros(4, np.float64)
    for r in res.results:
        tot += r["partial"].astype(np.float64).sum(axis=0)
    loss = (tot[0] + tot[2] + tot[3]) / (tot[1] + 1e-6)
    return np.array(loss, dtype=np.float32)
